# revision 3
# baseline (speedup 1.0000x reference)
"""Trainium2 Bass kernel for nn_Attention_72791105732908 (sparse_attention).

Reference computation (L=2048, B=64, H=1024, HC=1024):
    outs   = prev_layer_outputs.transpose(1, 0, 2)              # [B, L, H]
    energy = tanh(concat([hidden_bcast, outs], -1) @ W_e.T + b_e)  # [B, L, HC]
    attn   = energy @ W_v                                        # [B, L]
    attn   = where(mask == 0, -1e10, attn); softmax over L
    out    = einsum('bl,blh->bh', attn, outs)[None]              # [1, B, H]

Strategy:
  - Data-parallel over batch: core i handles batches 8i..8i+7. No collectives.
  - Split the concat matmul: q[b] = hidden[b] @ W_h.T + b_e is computed once
    per batch (tiny), the big matmul is outs @ W_o.T (halves the FLOPs).
  - bf16 on the PE for the big matmuls (fp32 PSUM accumulation).
  - outs arrives [L, b, H]; the energy matmul contracts over H, so outs is
    transposed to [H, L] tiles with the DMA xbar (2-byte dtype, DRAM->SBUF,
    mapping T[p, j, l] = outs[l, 128j + p]). Host pre-permutes W to match.
  - Masked softmax without max-subtraction (scores are bounded: |s| <= 32):
    w = exp(s) * mask; normalization folded into the output scale.
  - Score row [1, 2048] is moved onto partitions via K=1 matmuls with a
    ones [1, 1] rhs (exact), giving lhsT columns for the weighted sum.
"""
import numpy as np
import ml_dtypes

import concourse.bacc as bacc
import concourse.mybir as mybir
import concourse.tile as tile
from concourse.bass_utils import run_bass_kernel_spmd

dt = mybir.dt
AF = mybir.ActivationFunctionType

L, B, H, HC = 2048, 64, 1024, 1024
NCORES = 8
BPC = B // NCORES        # batches per core
P = 128
LC = L // P              # 16 l-chunks
JH = H // P              # 8 h-chunks
MC = HC // P             # 8 c-chunks
L4 = L // 512            # 4 chunks of 512 along L
BF = ml_dtypes.bfloat16

_CACHE = {}


def _build():
    nc = bacc.Bacc()
    prev = nc.dram_tensor("prev", [L, BPC, H], dt.bfloat16, kind="ExternalInput")
    WoT = nc.dram_tensor("WoT", [P, JH, HC], dt.bfloat16, kind="ExternalInput")
    WhT = nc.dram_tensor("WhT", [P, JH, HC], dt.bfloat16, kind="ExternalInput")
    hT = nc.dram_tensor("hT", [P, JH, BPC], dt.bfloat16, kind="ExternalInput")
    WvT = nc.dram_tensor("WvT", [P, MC], dt.bfloat16, kind="ExternalInput")
    beT = nc.dram_tensor("beT", [P, MC], dt.float32, kind="ExternalInput")
    mskT = nc.dram_tensor("mskT", [P, BPC, LC], dt.float32, kind="ExternalInput")
    out = nc.dram_tensor("out", [BPC, H], dt.float32, kind="ExternalOutput")

    with tile.TileContext(nc) as tc:
        with (
            tc.tile_pool(name="const", bufs=1) as const,
            tc.tile_pool(name="data", bufs=2) as data,
            tc.tile_pool(name="et", bufs=3) as etp,
            tc.tile_pool(name="small", bufs=2) as small,
            tc.tile_pool(name="pse", bufs=2, space="PSUM") as pse_p,
            tc.tile_pool(name="pss", bufs=2, space="PSUM") as pss_p,
            tc.tile_pool(name="psw", bufs=2, space="PSUM") as psw_p,
            tc.tile_pool(name="pwo", bufs=2, space="PSUM") as pwo_p,
        ):
            # ---- constants
            wo = const.tile([P, JH, HC], dt.bfloat16)
            nc.sync.dma_start(out=wo[:], in_=WoT[:])
            wh = const.tile([P, JH, HC], dt.bfloat16)
            nc.sync.dma_start(out=wh[:], in_=WhT[:])
            ht = const.tile([P, JH, BPC], dt.bfloat16)
            nc.sync.dma_start(out=ht[:], in_=hT[:])
            wv = const.tile([P, MC], dt.bfloat16)
            nc.sync.dma_start(out=wv[:], in_=WvT[:])
            be = const.tile([P, MC], dt.float32)
            nc.sync.dma_start(out=be[:], in_=beT[:])
            mk = const.tile([P, BPC, LC], dt.float32)
            nc.sync.dma_start(out=mk[:], in_=mskT[:])
            ones1 = const.tile([1, 1], dt.float32)
            nc.vector.memset(ones1[:], 1.0)
            onesp = const.tile([P, 1], dt.bfloat16)
            nc.vector.memset(onesp[:], 1.0)

            # ---- q[b, c] = hidden[b] @ W_h.T + b_e, laid out [c-part, m, b]
            qb = const.tile([P, MC, BPC], dt.float32)
            for m in range(MC):
                psq = pse_p.tile([P, 512], dt.float32, tag="pse")
                for u in range(JH):
                    nc.tensor.matmul(
                        psq[:, :BPC],
                        wh[:, u, m * P:(m + 1) * P],
                        ht[:, u, :],
                        start=(u == 0), stop=(u == JH - 1),
                    )
                nc.vector.tensor_scalar_add(qb[:, m, :], psq[:, :BPC], be[:, m:m + 1])

            # ---- per-batch pipeline
            for b in range(BPC):
                # transposed activations: T[p, j, l] = outs[l, 128j + p]
                tb = data.tile([P, JH, L], dt.bfloat16, tag="tb")
                for c in range(LC):
                    nc.sync.dma_start(
                        out=tb[:, :, c * P:(c + 1) * P],
                        in_=prev[c * P:(c + 1) * P, b, :],
                        transpose=True,
                    )
                # natural activations for the weighted sum: nat[p, c, h] = outs[128c + p, h]
                nat = data.tile([P, LC, H], dt.bfloat16, tag="nat")
                nc.sync.dma_start(
                    out=nat[:],
                    in_=prev[:, b, :].rearrange("(c p) h -> p c h", p=P),
                )

                es = small.tile([1, L], dt.float32, tag="es")
                for l4 in range(L4):
                    lsl = slice(l4 * 512, (l4 + 1) * 512)
                    pss = pss_p.tile([1, 512], dt.float32, tag="pss")
                    for m in range(MC):
                        pse = pse_p.tile([P, 512], dt.float32, tag="pse")
                        for j in range(JH):
                            nc.tensor.matmul(
                                pse[:],
                                wo[:, j, m * P:(m + 1) * P],
                                tb[:, j, lsl],
                                start=(j == 0), stop=(j == JH - 1),
                            )
                        et = etp.tile([P, 512], dt.bfloat16, tag="et")
                        nc.scalar.activation(et[:], pse[:], AF.Tanh, bias=qb[:, m, b:b + 1])
                        nc.tensor.matmul(
                            pss[:],
                            wv[:, m:m + 1],
                            et[:],
                            start=(m == 0), stop=(m == MC - 1),
                        )
                    nc.scalar.activation(es[0:1, lsl], pss[:], AF.Exp)

                # move scores onto partitions: wT[p, c] = es[128c + p] (K=1 matmuls, exact)
                psw = psw_p.tile([P, LC], dt.float32, tag="psw")
                for c in range(LC):
                    nc.tensor.matmul(
                        psw[:, c:c + 1],
                        es[0:1, c * P:(c + 1) * P],
                        ones1[:],
                        start=True, stop=True,
                    )
                wtf = small.tile([P, LC], dt.float32, tag="wtf")
                nc.vector.tensor_mul(wtf[:], psw[:], mk[:, b, :])
                wt = small.tile([P, LC], dt.bfloat16, tag="wt")
                nc.vector.tensor_copy(wt[:], wtf[:])

                # sum of weights (of the bf16-rounded values actually used)
                pssum = psw_p.tile([1, LC], dt.float32, tag="psw")
                nc.tensor.matmul(pssum[:], onesp[:], wt[:], start=True, stop=True)
                ssum = small.tile([1, 1], dt.float32, tag="ssum")
                nc.vector.reduce_sum(ssum[:], pssum[:], axis=mybir.AxisListType.X)
                rsum = small.tile([1, 1], dt.float32, tag="rsum")
                nc.vector.reciprocal(rsum[:], ssum[:])

                # weighted sum over L, normalized by scale on the way out
                ob = small.tile([1, H], dt.float32, tag="ob")
                for half in range(2):
                    hsl = slice(half * 512, (half + 1) * 512)
                    pwo = pwo_p.tile([1, 512], dt.float32, tag="pwo")
                    for c in range(LC):
                        nc.tensor.matmul(
                            pwo[:],
                            wt[:, c:c + 1],
                            nat[:, c, hsl],
                            start=(c == 0), stop=(c == LC - 1),
                        )
                    nc.scalar.activation(ob[0:1, hsl], pwo[:], AF.Copy, scale=rsum[0:1, :])
                nc.sync.dma_start(out=out[b:b + 1, :], in_=ob[:])

    nc.finalize()
    return nc


def _in_maps(prev_layer_outputs, hidden, mask, W_e, b_e, W_v):
    # host-side layout prep (cheap, O(MB) except the bf16 cast of prev)
    WoT = np.ascontiguousarray(
        W_e[:, H:].T.reshape(JH, P, HC).transpose(1, 0, 2)).astype(BF)
    WhT = np.ascontiguousarray(
        W_e[:, :H].T.reshape(JH, P, HC).transpose(1, 0, 2)).astype(BF)
    hT_full = np.ascontiguousarray(
        hidden.T.reshape(JH, P, B).transpose(1, 0, 2)).astype(BF)
    WvT = np.ascontiguousarray(W_v.reshape(MC, P).T).astype(BF)
    beT = np.ascontiguousarray(b_e.reshape(MC, P).T).astype(np.float32)

    in_maps = []
    for i in range(NCORES):
        bs = slice(i * BPC, (i + 1) * BPC)
        prev_i = np.ascontiguousarray(prev_layer_outputs[:, bs, :]).astype(BF)
        mskT_i = np.ascontiguousarray(
            mask[bs, :].reshape(BPC, LC, P).transpose(2, 0, 1)).astype(np.float32)
        hT_i = np.ascontiguousarray(hT_full[:, :, bs])
        in_maps.append({
            "prev": prev_i, "WoT": WoT, "WhT": WhT, "hT": hT_i,
            "WvT": WvT, "beT": beT, "mskT": mskT_i,
        })
    return in_maps


def kernel(prev_layer_outputs, hidden, mask, W_e, b_e, W_v):
    if "nc" not in _CACHE:
        _CACHE["nc"] = _build()
    nc = _CACHE["nc"]
    in_maps = _in_maps(prev_layer_outputs, hidden, mask, W_e, b_e, W_v)
    res = run_bass_kernel_spmd(nc, in_maps, list(range(NCORES)))
    out = np.concatenate([np.asarray(r["out"]) for r in res.results], axis=0)
    return out[None, :, :].astype(np.float32)


def run_traced(inputs):
    """Profiled run (test harness only)."""
    if "nc" not in _CACHE:
        _CACHE["nc"] = _build()
    nc = _CACHE["nc"]
    in_maps = _in_maps(**inputs)
    return run_bass_kernel_spmd(nc, in_maps, list(range(NCORES)), trace=True)
